# revision 27
# baseline (speedup 1.0000x reference)
"""Call-guided sparse attention kernel for Trainium2 (8 NeuronCores).

Sharding: batch (4) x head-group (2 groups of 4 heads) -> 8 cores.

v2 design (cost-model driven):
  - all matmuls f16 (fp32 is 4x); matmul cost ~ moving columns only
  - per-head score/AV matmuls via 32-row partition bands; heads 2,3 use
    DMA-shifted _hi copies (PE base partition must be 0/32/64)
  - banded window attention (256-wide band, 2 j-subtiles per row tile)
    with ADDITIVE masks folded into the scores psum via identity-matmul
    (exp of masked entries underflows f16 to 0) - no mask multiply
  - V tiles embed a ones column per head ([32 v | 1]) so AV matmuls
    produce row sums for free (33-row out blocks at psum base 0/64);
    psum rows 33:64 are memset to 1.0 so one wide reciprocal over rows
    [32:97] stays finite, then selE65-matmul broadcasts the recips
  - caller rows (opcode==0, padded to NCAP=260) get dense union-masked
    attention; union mask = max(window01, sc >= top16 threshold) via
    max8/match_replace/max8, transposed on PE into [j, i] tiles
  - engine budget: Act = exp + some evictions, DVE = psum traffic +
    max8, Pool = SBUF-only elementwise (caller em, union, memsets)
"""

import os
import sys

import numpy as np

for _p in ("/opt/trn_rl_repo", "/root/.axon_site/_ro/trn_rl_repo"):
    if os.path.isdir(_p) and _p not in sys.path:
        sys.path.insert(0, _p)

import concourse.bass as bass
import concourse.mybir as mybir
from concourse import bacc
from concourse.tile import TileContext
from concourse.bass_utils import run_bass_kernel_spmd

F32 = mybir.dt.float32
F16 = mybir.dt.float16
AF = mybir.ActivationFunctionType
ALU = mybir.AluOpType

B, S, D, H = 4, 2048, 256, 8
DK = D // H          # 32
HPC = H // 2         # 4 heads per core
DH = HPC * DK        # 128 context dims per core
WINDOW = 50
NCAP = 260           # padded caller-row capacity (max actual count is 260)
SCALE = 1.0 / np.sqrt(np.float32(DK))
NT = S // 128        # 16 row tiles
NM = 3               # caller-row tiles: 128 + 128 + 4
MT_ROWS = (128, 128, NCAP - 256)
NEGM = -30.0         # additive mask: exp(-30) underflows f16 to 0


def _build_program(stage=4):
    nc = bacc.Bacc("TRN2", target_bir_lowering=False, debug=False,
                   num_devices=8)

    xTh = nc.dram_tensor("xTh", [2, 128, S], F16, kind="ExternalInput")
    xcTh = nc.dram_tensor("xcTh", [2, 128, NCAP], F16, kind="ExternalInput")
    wqc_d = nc.dram_tensor("wqc", [2, 128, D], F16, kind="ExternalInput")
    wk_d = nc.dram_tensor("wk", [2, 128, D], F16, kind="ExternalInput")
    wv_d = nc.dram_tensor("wv", [2, 128, HPC * 32], F16, kind="ExternalInput")
    woh_d = nc.dram_tensor("woh", [2, 97, D], F16, kind="ExternalInput")
    selE_d = nc.dram_tensor("selE", [97, 97], F16,
                        kind="ExternalInput")
    ident_d = nc.dram_tensor("ident", [128, 128], F16, kind="ExternalInput")
    mad_d = nc.dram_tensor("mad", [128, 4, HPC, 128], F16,
                           kind="ExternalInput")
    win_d = nc.dram_tensor("win", [NCAP, S], F16, kind="ExternalInput")
    outT = nc.dram_tensor("outT", [128, 2, S], F32, kind="ExternalOutput")
    outcT = nc.dram_tensor("outcT", [128, 2, NCAP], F32,
                           kind="ExternalOutput")

    with TileContext(nc) as tc:
        with (
            tc.tile_pool(name="const", bufs=1) as cst,
            tc.tile_pool(name="persist", bufs=1) as per,
        ):
            # ---------- constants ----------
            wqc = [cst.tile([128, D], F16, tag=f"wqc{k}", name=f"wqc{k}")
                   for k in range(2)]
            wk = [cst.tile([128, D], F16, tag=f"wk{k}", name=f"wk{k}")
                  for k in range(2)]
            wv = [cst.tile([128, HPC * 32], F16, tag=f"wv{k}", name=f"wv{k}")
                  for k in range(2)]
            for k in range(2):
                nc.sync.dma_start(wqc[k][:], wqc_d[k])
                nc.sync.dma_start(wk[k][:], wk_d[k])
                nc.sync.dma_start(wv[k][:], wv_d[k])
            woh = [cst.tile([97, D], F16, tag=f"woh{p}", name=f"woh{p}")
                   for p in range(2)]
            for p in range(2):
                nc.sync.dma_start(woh[p][:], woh_d[p])
            selE = cst.tile([97, 97], F16, tag="selE")
            nc.sync.dma_start(selE[:], selE_d[:])
            ident = cst.tile([128, 128], F16, tag="ident")
            nc.sync.dma_start(ident[:], ident_d[:])
            ones1 = cst.tile([1, 512], F16, tag="ones1")
            nc.vector.memset(ones1[:], 1.0)
            mad = cst.tile([128, 4, HPC, 128], F16, tag="mad")
            nc.sync.dma_start(mad[:], mad_d[:])
            win_sb = [cst.tile([128, S], F16, tag=f"win{m}", name=f"win{m}")
                      for m in range(NM)]
            for m in range(NM):
                nc.sync.dma_start(win_sb[m][0:MT_ROWS[m], :],
                                  win_d[m * 128:m * 128 + MT_ROWS[m], :])

            # ---------- persistent activations ----------
            xh = [per.tile([128, S], F16, tag=f"xh{k}", name=f"xh{k}")
                  for k in range(2)]
            xch = [per.tile([128, NCAP], F16, tag=f"xch{k}", name=f"xch{k}")
                   for k in range(2)]
            for k in range(2):
                nc.sync.dma_start(xh[k][:], xTh[k])
                nc.sync.dma_start(xch[k][:], xcTh[k])

            qn = per.tile([128, S], F16, tag="qn")
            kfth = per.tile([128, S], F16, tag="kfth")
            kft2 = per.tile([128, S], F16, tag="kft2")
            qc = [per.tile([128, NCAP], F16, tag=f"qc{m}", name=f"qc{m}")
                  for m in range(2)]
            # per-head base-0 band copies (mixing lhsT base partitions
            # between back-to-back matmuls faults the PE)
            qb_t = [per.tile([32, S], F16, tag=f"qb{h}", name=f"qb{h}")
                    for h in range(1, 4)]
            kb_t = [per.tile([32, S], F16, tag=f"kb{h}", name=f"kb{h}")
                    for h in range(1, 4)]
            qcb_t = [per.tile([32, NCAP], F16, tag=f"qcb{h}",
                              name=f"qcb{h}") for h in range(1, 4)]
            # V tiles [j, (4 heads x 33)] (col 32 of each head block = 1):
            # aligned a0..a15 and shifted t1..t16 ([128i-64, 128i+64))
            va = [per.tile([128, HPC, 33], F16, tag=f"va{j}", name=f"va{j}")
                  for j in range(NT)]
            vs = [per.tile([128, HPC, 33], F16, tag=f"vs{j}", name=f"vs{j}")
                  for j in range(1, NT + 1)]
            sc_sb = [per.tile([128, S], F32, tag=f"sc{m}", name=f"sc{m}")
                     for m in range(NM)]
            al_sb = [per.tile([128, S], F16, tag=f"al{m}", name=f"al{m}")
                     for m in range(NM)]
            alT_sb = [per.tile([128, 1, NCAP], F16, tag=f"alT{j}",
                               name=f"alT{j}") for j in range(NT)]

            with (
                tc.tile_pool(name="pmm", bufs=4, space="PSUM") as pmm,
                tc.tile_pool(name="wrk", bufs=3) as wrk,
            ):
                # ---------- projections ----------
                for name, dst, wsel, lo in (
                    ("q", qn, wqc, 0), ("k", kfth, wk, 0),
                    ("k2", kft2, wk, 128),
                ):
                    for c in range(4):
                        sl = bass.ts(c, 512)
                        ps = pmm.tile([128, 512], F32, tag="mm")
                        for k in range(2):
                            nc.tensor.matmul(ps[:], wsel[k][:, lo:lo + 128],
                                             xh[k][:, sl],
                                             start=(k == 0), stop=(k == 1))
                        nc.vector.tensor_copy(dst[:, sl], ps[:])
                for m in range(2):
                    ps = pmm.tile([128, 512], F32, tag="mm")
                    for k in range(2):
                        nc.tensor.matmul(ps[:, 0:NCAP],
                                         wqc[k][:, m * 128:(m + 1) * 128],
                                         xch[k][:], start=(k == 0),
                                         stop=(k == 1))
                    nc.vector.tensor_copy(qc[m][:], ps[:, 0:NCAP])
                # V tiles; ones cols via Pool memset (SBUF)
                for idx, (vt, j0) in enumerate(
                        [(va[j], j * 128) for j in range(NT)] +
                        [(vs[j - 1], j * 128 - 64) for j in range(1, NT + 1)]):
                    lo = max(j0, 0)
                    hi = min(j0 + 128, S)
                    p0 = lo - j0
                    rows = hi - lo
                    ps = pmm.tile([128, 512], F32, tag="mm")
                    for k in range(2):
                        nc.tensor.matmul(ps[0:rows, 0:128], xh[k][:, lo:hi],
                                         wv[k][:], start=(k == 0),
                                         stop=(k == 1))
                    dstv = vt[p0:p0 + rows, :, 0:32]
                    srcv = ps[0:rows, 0:128].rearrange("p (h n) -> p h n", h=4)
                    if idx % 2 == 0:
                        nc.vector.tensor_copy(dstv, srcv)
                    else:
                        nc.scalar.activation(dstv, srcv, AF.Copy)
                    nc.gpsimd.memset(vt[:, :, 32:33], 1.0)

                for h in range(1, 4):
                    b0 = h * 32
                    nc.sync.dma_start(qb_t[h - 1][:], qn[b0:b0 + 32, :])
                    nc.sync.dma_start(kb_t[h - 1][:], kfth[b0:b0 + 32, :])
                    nc.sync.dma_start(qcb_t[h - 1][:],
                                      qc[0][b0:b0 + 32, :])

            if stage == 1:
                with tc.tile_pool(name="stub", bufs=1) as stub:
                    z = stub.tile([128, 2, S], F32, tag="z")
                    nc.vector.memset(z[:], 0.0)
                    nc.sync.dma_start(outT[:], z[:])

            # ---------- banded attention + interleaved routing ----------
            if stage >= 2:
             with (
                tc.tile_pool(name="pband", bufs=1, space="PSUM") as pb,
                tc.tile_pool(name="bwk", bufs=3) as bwk,
                tc.tile_pool(name="rwk", bufs=1) as rwk,
             ):
                def routing_scores(mt, c):
                    rows = MT_ROWS[mt]
                    msl = slice(mt * 128, mt * 128 + rows)
                    sl = bass.ts(c, 512)
                    ps = pb.tile([128, 512], F32, tag="rt", bufs=1)
                    nc.tensor.matmul(ps[0:rows, :], qc[0][:, msl],
                                     kfth[:, sl], start=True, stop=False)
                    nc.tensor.matmul(ps[0:rows, :], qc[1][:, msl],
                                     kft2[:, sl], start=False, stop=True)
                    nc.scalar.activation(sc_sb[mt][0:rows, sl],
                                         ps[0:rows, :], AF.Copy)

                def routing_thresh(mt):
                    rows = MT_ROWS[mt]
                    m8a = rwk.tile([128, 8], F32, tag="m8a")
                    m8b = rwk.tile([128, 8], F32, tag="m8b")
                    tmp = rwk.tile([128, S], F32, tag="mrtmp")
                    nc.vector.max(out=m8a[0:rows, :], in_=sc_sb[mt][0:rows, :])
                    nc.vector.match_replace(out=tmp[0:rows, :],
                                            in_to_replace=m8a[0:rows, :],
                                            in_values=sc_sb[mt][0:rows, :],
                                            imm_value=-1e30)
                    nc.vector.max(out=m8b[0:rows, :], in_=tmp[0:rows, :])
                    nc.vector.scalar_tensor_tensor(
                        out=al_sb[mt][0:rows, :], in0=sc_sb[mt][0:rows, :],
                        scalar=m8b[0:rows, 7:8], in1=win_sb[mt][0:rows, :],
                        op0=ALU.is_ge, op1=ALU.max)
                    nc.vector.tensor_scalar(al_sb[mt][0:rows, :],
                                            al_sb[mt][0:rows, :], 30.0,
                                            -30.0, op0=ALU.mult, op1=ALU.add)

                def routing_transpose(jt):
                    jsl = bass.ts(jt, 128)
                    ps = pb.tile([128, 1024], F16, tag="tr", bufs=1)
                    for mt in range(NM):
                        nc.tensor.transpose(ps[:, mt * 128:(mt + 1) * 128],
                                            al_sb[mt][:, jsl], ident[:])
                    if jt % 2 == 0:
                        nc.vector.tensor_copy(alT_sb[jt][:, 0, :],
                                              ps[:, 0:NCAP])
                    else:
                        nc.scalar.activation(alT_sb[jt][:, 0, :],
                                             ps[:, 0:NCAP], AF.Copy)

                routing_steps = (
                    [lambda m=m, c=c: routing_scores(m, c)
                     for m in range(NM) for c in range(4)] +
                    [lambda m=m: routing_thresh(m) for m in range(NM)] +
                    [lambda j=j: routing_transpose(j) for j in range(NT)]
                ) if stage >= 3 else []
                rstep = iter(routing_steps)

                bpart = int(os.environ.get("CGSA_BPART", "4"))
                for it in range(int(os.environ.get("CGSA_NT", NT))):
                    r0 = it * 128
                    isl = bass.ts(it, 128)
                    if it == 0:
                        subs = [(va[0], 0, 0, 128), (va[1], 128, 1, 128)]
                    elif it == NT - 1:
                        subs = [(vs[it - 1], r0 - 64, 2, 128),
                                (vs[it], r0 + 64, 3, 64)]
                    else:
                        subs = [(vs[it - 1], r0 - 64, 2, 128),
                                (vs[it], r0 + 64, 3, 128)]

                    av = pb.tile([128, 2, 256], F32, tag="av", bufs=2)
                    if True:
                        nc.tensor.matmul(
                            av[32:64, :, :].rearrange("p a b -> p (a b)"),
                            ones1[:, 0:32], ones1[:, 0:512],
                            start=True, stop=True, skip_group_check=True)
                    nsub = len(subs)
                    for si, (vt, j0, mc, jw) in enumerate(subs):
                        ps = pb.tile([128, HPC, 128], F32, tag="sc", bufs=2)
                        for h in range(HPC):
                            kt = kfth[0:32, :] if h == 0 else kb_t[h - 1][:]
                            qt = qn[0:32, :] if h == 0 else qb_t[h - 1][:]
                            nc.tensor.matmul(
                                ps[0:jw, h, :], kt[:, j0:j0 + jw],
                                qt[:, isl], start=(h == 0),
                                stop=False, skip_group_check=True)
                        nc.tensor.matmul(
                            ps[0:jw, :, :].rearrange("p a b -> p (a b)"),
                            ident[:, 0:jw],
                            mad[:, mc, :, :].rearrange("p a b -> p (a b)"),
                            start=False, stop=True, skip_group_check=True)
                        if bpart < 2:
                            continue
                        e = bwk.tile([128, HPC, 128], F16, tag="be")
                        nc.scalar.activation(e[0:jw, :, :], ps[0:jw, :, :],
                                             AF.Exp)
                        st = (si == 0)
                        sp = (si == nsub - 1)
                        for h in range(HPC):
                            p = h // 2
                            ob = (h % 2) * 64
                            nc.tensor.matmul(
                                av[ob:ob + 33, p, 0:128], vt[0:jw, h, :],
                                e[0:jw, h, :], start=(st and h < 2),
                                stop=(sp and h >= 2), skip_group_check=True)

                    if bpart < 3:
                        continue
                    r2i = bwk.tile([97, 2, 128], F16, tag="br2i")
                    with nc.allow_low_precision(reason="f16 recip bcast"):
                        nc.vector.reciprocal(r2i[:], av[0:97, :, 0:128])
                    r2w = bwk.tile([97, 2, 128], F16, tag="br2")
                    nc.vector.tensor_scalar(r2w[:], r2i[:], 60000.0, -60000.0,
                                            op0=ALU.min, op1=ALU.max)
                    rbp = pb.tile([97, 2, 256], F32, tag="rb", bufs=1)
                    for p in range(2):
                        nc.tensor.matmul(rbp[:, p, 0:128], selE[:],
                                         r2w[:, p, :], start=True, stop=True)
                    rbs = bwk.tile([97, 2, 128], F16, tag="brbs")
                    if it % 2 == 0:
                        nc.scalar.activation(rbs[:], rbp[:, :, 0:128],
                                             AF.Copy)
                    else:
                        nc.vector.tensor_copy(rbs[:], rbp[:, :, 0:128])
                    ctx = bwk.tile([97, 2, 128], F16, tag="bctx")
                    nc.vector.tensor_tensor(ctx[:], av[0:97, :, 0:128],
                                            rbs[:], op=ALU.mult)
                    pso = pb.tile([128, 2, 256], F32, tag="out", bufs=1)
                    for m in range(2):
                        msl = bass.ts(m, 128)
                        nc.tensor.matmul(pso[:, m, 0:128], woh[0][:, msl],
                                         ctx[:, 0, :], start=True, stop=False)
                        nc.tensor.matmul(pso[:, m, 0:128], woh[1][:, msl],
                                         ctx[:, 1, :], start=False, stop=True)
                    if bpart >= 4:
                        osb = bwk.tile([128, 2, 128], F32, tag="osb")
                        nc.scalar.activation(osb[:], pso[:, :, 0:128],
                                             AF.Copy)
                        nc.scalar.dma_start(outT[:, :, r0:r0 + 128], osb[:])

                    for _ in range(2):
                        step = next(rstep, None)
                        if step is not None:
                            step()
                for step in rstep:
                    step()

            # ---------- caller dense attention ----------
            if stage < 4:
                with tc.tile_pool(name="stub2", bufs=1) as stub2:
                    zc = stub2.tile([128, 2, NCAP], F32, tag="zc")
                    nc.vector.memset(zc[:], 0.0)
                    nc.sync.dma_start(outcT[:], zc[:])
            elif True:
             with tc.tile_pool(name="cacc", bufs=1, space="PSUM") as cacc:
                pCX = cacc.tile([128, 512], F32, tag="cavX")
                pCY = cacc.tile([128, 512], F32, tag="cavY")
                with (
                    tc.tile_pool(name="cps", bufs=2, space="PSUM") as cps,
                    tc.tile_pool(name="cwk", bufs=3) as cwk,
                ):
                    nc.tensor.matmul(pCX[32:64, :], ones1[:, 0:32],
                                     ones1[:, 0:512], start=True, stop=True,
                                     skip_group_check=True)
                    nc.tensor.matmul(pCY[32:64, :], ones1[:, 0:32],
                                     ones1[:, 0:512], start=True, stop=True,
                                     skip_group_check=True)
                    for jt in range(NT):
                        jsl = bass.ts(jt, 128)
                        psA = cps.tile([128, 2, 512], F32, tag="cscA",
                                       bufs=1)
                        psB = cps.tile([128, 2, 512], F32, tag="cscB",
                                       bufs=1)
                        for h in range(HPC):
                            kt = kfth[0:32, :] if h == 0 else kb_t[h - 1][:]
                            qt = (qc[0][0:32, :] if h == 0
                                  else qcb_t[h - 1][:])
                            psh = psA if h < 2 else psB
                            nc.tensor.matmul(psh[:, h % 2, 0:NCAP],
                                             kt[:, jsl], qt[:],
                                             start=True, stop=False,
                                             skip_group_check=True)
                            nc.tensor.matmul(psh[:, h % 2, 0:NCAP],
                                             ident[:], alT_sb[jt][:, 0, :],
                                             start=False, stop=True,
                                             skip_group_check=True)
                        eA = cwk.tile([128, 2, NCAP], F16, tag="ceA")
                        eB = cwk.tile([128, 2, NCAP], F16, tag="ceB")
                        nc.scalar.activation(eA[:], psA[:, :, 0:NCAP], AF.Exp)
                        nc.scalar.activation(eB[:], psB[:, :, 0:NCAP], AF.Exp)
                        st = (jt == 0)
                        sp = (jt == NT - 1)
                        for h in range(HPC):
                            pst = pCX if h < 2 else pCY
                            eh = eA if h < 2 else eB
                            ob = (h % 2) * 64
                            nc.tensor.matmul(pst[ob:ob + 33, 0:NCAP],
                                             va[jt][:, h, :],
                                             eh[:, h % 2, :],
                                             start=st, stop=sp,
                                             skip_group_check=True)

                with (
                    tc.tile_pool(name="cep", bufs=1, space="PSUM") as cep,
                    tc.tile_pool(name="cwk2", bufs=1) as cwk2,
                ):
                    rcX = cwk2.tile([97, NCAP], F16, tag="crcX")
                    rcY = cwk2.tile([97, NCAP], F16, tag="crcY")
                    ri = cwk2.tile([97, 2, NCAP], F16, tag="cri")
                    with nc.allow_low_precision(reason="f16 recip bcast"):
                        nc.vector.reciprocal(ri[:, 0, :], pCX[0:97, 0:NCAP])
                        nc.vector.reciprocal(ri[:, 1, :], pCY[0:97, 0:NCAP])
                    nc.vector.tensor_scalar(rcX[:], ri[:, 0, :], 60000.0,
                                            -60000.0, op0=ALU.min,
                                            op1=ALU.max)
                    nc.vector.tensor_scalar(rcY[:], ri[:, 1, :], 60000.0,
                                            -60000.0, op0=ALU.min,
                                            op1=ALU.max)
                    rbc = cep.tile([97, 2, 512], F32, tag="crb")
                    nc.tensor.matmul(rbc[:, 0, 0:NCAP], selE[:], rcX[:],
                                     start=True, stop=True)
                    nc.tensor.matmul(rbc[:, 1, 0:NCAP], selE[:], rcY[:],
                                     start=True, stop=True)
                    rbcs = cwk2.tile([97, 2, NCAP], F16, tag="crbs")
                    nc.scalar.activation(rbcs[:], rbc[:, :, 0:NCAP], AF.Copy)
                    ctxX = cwk2.tile([97, NCAP], F16, tag="cctxX")
                    ctxY = cwk2.tile([97, NCAP], F16, tag="cctxY")
                    nc.vector.tensor_tensor(ctxX[:], pCX[0:97, 0:NCAP],
                                            rbcs[:, 0, :], op=ALU.mult)
                    nc.vector.tensor_tensor(ctxY[:], pCY[0:97, 0:NCAP],
                                            rbcs[:, 1, :], op=ALU.mult)
                    psoc = cep.tile([128, 2, 512], F32, tag="cout")
                    for m in range(2):
                        msl = bass.ts(m, 128)
                        nc.tensor.matmul(psoc[:, m, 0:NCAP], woh[0][:, msl],
                                         ctxX[:], start=True, stop=False)
                        nc.tensor.matmul(psoc[:, m, 0:NCAP], woh[1][:, msl],
                                         ctxY[:], start=False, stop=True)
                    osc = cwk2.tile([128, 2, NCAP], F32, tag="osc")
                    nc.vector.tensor_copy(osc[:], psoc[:, :, 0:NCAP])
                    nc.sync.dma_start(outcT[:], osc[:])

    nc.compile()
    nc.finalize()
    return nc


_NC_CACHE = None


def _get_program():
    global _NC_CACHE
    if _NC_CACHE is None:
        _NC_CACHE = _build_program(
            int(os.environ.get("CGSA_STAGE", "4")))
    return _NC_CACHE


def _np_fallback(x, Wq, bq, Wk, bk, Wv, bv, Wo, bo, opcode_types, pad_mask):
    """Exact numpy port of the reference (slow; only for inputs the fast
    path does not support: nonzero biases or non-trivial pad_mask)."""
    x = np.asarray(x, np.float32)
    b, s, d = x.shape
    scale = np.float32(SCALE)
    TOPK = 16

    def heads(t):
        return t.reshape(b, s, H, DK).transpose(0, 2, 1, 3)

    Q = heads((x @ Wq + bq).astype(np.float32))
    K = heads((x @ Wk + bk).astype(np.float32))
    V = heads((x @ Wv + bv).astype(np.float32))
    pos = np.arange(s)
    window = np.abs(pos[:, None] - pos[None, :]) <= WINDOW
    ms = np.einsum("bhid,bhjd->bij", Q, K).astype(np.float32) * (scale / H)
    topk_idx = np.argsort(-ms, axis=-1, kind="stable")[:, :, :TOPK]
    guided = np.zeros((b, s, s), bool)
    guided[np.arange(b)[:, None, None], np.arange(s)[None, :, None],
           topk_idx] = True
    attn = window[None] | (guided & (np.asarray(opcode_types) == 0)[:, :, None])
    out = np.empty((b, s, d), np.float32)
    for bb in range(b):
        ctx = np.empty((H, s, DK), np.float32)
        for h in range(H):
            sco = (Q[bb, h] @ K[bb, h].T) * scale
            sco = np.where(attn[bb], sco, -1e9)
            sco = np.where(np.asarray(pad_mask)[bb][None, :] != 0, sco, -1e9)
            e = np.exp(sco - sco.max(axis=-1, keepdims=True))
            w = np.nan_to_num(e / e.sum(axis=-1, keepdims=True))
            ctx[h] = w @ V[bb, h]
        out[bb] = ctx.transpose(1, 0, 2).reshape(s, d)
    return out @ Wo + bo


def _host_prepare(x, Wq, Wk, Wv, Wo, opcode_types):
    x = np.ascontiguousarray(np.asarray(x, np.float32))
    Wq = np.asarray(Wq, np.float32) * SCALE
    Wk = np.asarray(Wk, np.float32)
    Wv = np.asarray(Wv, np.float32)
    Wo = np.asarray(Wo, np.float32)
    opcode = np.asarray(opcode_types)

    # additive banded masks [p_j, class, q_i]; class offsets j0-r0:
    # 0 -> 0, 1 -> +128, 2 -> -64, 3 -> +64
    mad = np.full((128, 4, HPC, 128), NEGM, np.float16)
    pj = np.arange(128)[:, None]
    qi = np.arange(128)[None, :]
    for c, off in enumerate((0, 128, -64, 64)):
        m = np.where(np.abs(off + pj - qi) <= WINDOW, np.float16(0.0),
                     np.float16(NEGM))
        mad[:, c, :, :] = m[:, None, :]

    selE = np.zeros((97, 97), np.float16)
    selE[32, 0:32] = 1.0
    selE[96, 64:96] = 1.0
    ident = np.eye(128, dtype=np.float16)

    in_maps = []
    meta = []
    for b in range(B):
        cidx = np.where(opcode[b] == 0)[0]
        nrows = len(cidx)
        if nrows > NCAP:
            raise RuntimeError(f"caller rows {nrows} exceed capacity {NCAP}")
        xc = np.zeros((NCAP, D), np.float32)
        xc[:nrows] = x[b, cidx]
        ci = np.full((NCAP,), -1e6, np.float64)
        ci[:nrows] = cidx
        win = (np.abs(ci[:, None] - np.arange(S)[None, :]) <= WINDOW)
        win = win.astype(np.float16)
        meta.append((cidx, nrows))
        for hg in range(2):
            own = np.arange(hg * DH, (hg + 1) * DH)
            rest = np.setdiff1d(np.arange(D), own)
            perm = np.concatenate([own, rest])
            woh_arr = np.zeros((2, 97, D), np.float32)
            for p in range(2):
                woh_arr[p, 0:32] = Wo[own[p * 64:p * 64 + 32]]
                woh_arr[p, 64:96] = Wo[own[p * 64 + 32:p * 64 + 64]]
            in_maps.append({
                "xTh": np.ascontiguousarray(
                    x[b].T.reshape(2, 128, S).astype(np.float16)),
                "xcTh": np.ascontiguousarray(
                    xc.T.reshape(2, 128, NCAP).astype(np.float16)),
                "wqc": np.ascontiguousarray(
                    Wq[:, perm].reshape(2, 128, D).astype(np.float16)),
                "wk": np.ascontiguousarray(
                    Wk[:, perm].reshape(2, 128, D).astype(np.float16)),
                "wv": np.ascontiguousarray(
                    Wv[:, own].reshape(2, 128, HPC * 32).astype(np.float16)),
                "woh": woh_arr.astype(np.float16),
                "selE": selE,
                "ident": ident,
                "mad": mad,
                "win": win,
            })
    return in_maps, meta


def _assemble(results, meta, bo):
    bo = np.asarray(bo, np.float32)
    out = np.empty((B, S, D), np.float32)
    for b in range(B):
        cidx, nrows = meta[b]
        full = (results[2 * b]["outT"] + results[2 * b + 1]["outT"])
        full = full.transpose(1, 0, 2).reshape(D, S).T
        if nrows > 0:
            oc = (results[2 * b]["outcT"] + results[2 * b + 1]["outcT"])
            oc = oc.transpose(1, 0, 2).reshape(D, NCAP).T[:nrows]
            full = full.copy()
            full[cidx] = oc
        out[b] = full + bo[None, :]
    return out


def kernel(x, Wq, bq, Wk, bk, Wv, bv, Wo, bo, opcode_types, pad_mask,
           _trace=False):
    if (np.any(np.asarray(bq)) or np.any(np.asarray(bk)) or
            np.any(np.asarray(bv)) or np.any(np.asarray(pad_mask) == 0)):
        return _np_fallback(x, Wq, bq, Wk, bk, Wv, bv, Wo, bo,
                            opcode_types, pad_mask)
    nc = _get_program()
    in_maps, meta = _host_prepare(x, Wq, Wk, Wv, Wo, opcode_types)
    res = run_bass_kernel_spmd(nc, in_maps, core_ids=list(range(8)),
                               trace=_trace)
    out = _assemble(res.results, meta, bo)
    if _trace:
        kernel.last_exec_time_ns = res.exec_time_ns
        kernel.last_results = res
    return out


# revision 28
# speedup vs baseline: 1.0906x; 1.0906x over previous
"""Call-guided sparse attention kernel for Trainium2 (8 NeuronCores).

Sharding: batch (4) x head-group (2 groups of 4 heads) -> 8 cores.

v2 design (cost-model driven):
  - all matmuls f16 (fp32 is 4x); matmul cost ~ moving columns only
  - per-head score/AV matmuls via 32-row partition bands; heads 2,3 use
    DMA-shifted _hi copies (PE base partition must be 0/32/64)
  - banded window attention (256-wide band, 2 j-subtiles per row tile)
    with ADDITIVE masks folded into the scores psum via identity-matmul
    (exp of masked entries underflows f16 to 0) - no mask multiply
  - V tiles embed a ones column per head ([32 v | 1]) so AV matmuls
    produce row sums for free (33-row out blocks at psum base 0/64);
    psum rows 33:64 are memset to 1.0 so one wide reciprocal over rows
    [32:97] stays finite, then selE65-matmul broadcasts the recips
  - caller rows (opcode==0, padded to NCAP=260) get dense union-masked
    attention; union mask = max(window01, sc >= top16 threshold) via
    max8/match_replace/max8, transposed on PE into [j, i] tiles
  - engine budget: Act = exp + some evictions, DVE = psum traffic +
    max8, Pool = SBUF-only elementwise (caller em, union, memsets)
"""

import os
import sys

import numpy as np

for _p in ("/opt/trn_rl_repo", "/root/.axon_site/_ro/trn_rl_repo"):
    if os.path.isdir(_p) and _p not in sys.path:
        sys.path.insert(0, _p)

import concourse.bass as bass
import concourse.mybir as mybir
from concourse import bacc
from concourse.tile import TileContext
from concourse.bass_utils import run_bass_kernel_spmd

F32 = mybir.dt.float32
F16 = mybir.dt.float16
AF = mybir.ActivationFunctionType
ALU = mybir.AluOpType

B, S, D, H = 4, 2048, 256, 8
DK = D // H          # 32
HPC = H // 2         # 4 heads per core
DH = HPC * DK        # 128 context dims per core
WINDOW = 50
NCAP = 260           # padded caller-row capacity (max actual count is 260)
SCALE = 1.0 / np.sqrt(np.float32(DK))
NT = S // 128        # 16 row tiles
NM = 3               # caller-row tiles: 128 + 128 + 4
MT_ROWS = (128, 128, NCAP - 256)
NEGM = -30.0         # additive mask: exp(-30) underflows f16 to 0


def _build_program(stage=4):
    nc = bacc.Bacc("TRN2", target_bir_lowering=False, debug=False,
                   num_devices=8)

    xTh = nc.dram_tensor("xTh", [2, 128, S], F16, kind="ExternalInput")
    xcTh = nc.dram_tensor("xcTh", [2, 128, NCAP], F16, kind="ExternalInput")
    wqc_d = nc.dram_tensor("wqc", [2, 128, D], F16, kind="ExternalInput")
    wk_d = nc.dram_tensor("wk", [2, 128, D], F16, kind="ExternalInput")
    wv_d = nc.dram_tensor("wv", [2, 128, HPC * 32], F16, kind="ExternalInput")
    woh_d = nc.dram_tensor("woh", [2, 97, D], F16, kind="ExternalInput")
    selE_d = nc.dram_tensor("selE", [97, 97], F16,
                        kind="ExternalInput")
    ident_d = nc.dram_tensor("ident", [128, 128], F16, kind="ExternalInput")
    mad_d = nc.dram_tensor("mad", [128, 4, HPC, 128], F16,
                           kind="ExternalInput")
    win_d = nc.dram_tensor("win", [NCAP, S], F16, kind="ExternalInput")
    outT = nc.dram_tensor("outT", [128, 2, S], F32, kind="ExternalOutput")
    outcT = nc.dram_tensor("outcT", [128, 2, NCAP], F32,
                           kind="ExternalOutput")

    with TileContext(nc) as tc:
        with (
            tc.tile_pool(name="const", bufs=1) as cst,
            tc.tile_pool(name="persist", bufs=1) as per,
        ):
            # ---------- constants ----------
            wqc = [cst.tile([128, D], F16, tag=f"wqc{k}", name=f"wqc{k}")
                   for k in range(2)]
            wk = [cst.tile([128, D], F16, tag=f"wk{k}", name=f"wk{k}")
                  for k in range(2)]
            wv = [cst.tile([128, HPC * 32], F16, tag=f"wv{k}", name=f"wv{k}")
                  for k in range(2)]
            pass
            woh = [cst.tile([97, D], F16, tag=f"woh{p}", name=f"woh{p}")
                   for p in range(2)]
            for p in range(2):
                nc.sync.dma_start(woh[p][:], woh_d[p])
            selE = cst.tile([97, 97], F16, tag="selE")
            nc.sync.dma_start(selE[:], selE_d[:])
            ident = cst.tile([128, 128], F16, tag="ident")
            nc.sync.dma_start(ident[:], ident_d[:])
            ones1 = cst.tile([1, 512], F16, tag="ones1")
            nc.vector.memset(ones1[:], 1.0)
            mad = cst.tile([128, 4, HPC, 128], F16, tag="mad")
            nc.sync.dma_start(mad[:], mad_d[:])
            win_sb = [cst.tile([128, S], F16, tag=f"win{m}", name=f"win{m}")
                      for m in range(NM)]

            # ---------- persistent activations ----------
            xh = [per.tile([128, S], F16, tag=f"xh{k}", name=f"xh{k}")
                  for k in range(2)]
            xch = [per.tile([128, NCAP], F16, tag=f"xch{k}", name=f"xch{k}")
                   for k in range(2)]
            for k in range(2):
                nc.sync.dma_start(xh[k][:], xTh[k])
                nc.sync.dma_start(wqc[k][:], wqc_d[k])
                nc.sync.dma_start(wk[k][:], wk_d[k])
                nc.sync.dma_start(wv[k][:], wv_d[k])
            for k in range(2):
                nc.sync.dma_start(xch[k][:], xcTh[k])
            for m in range(NM):
                nc.scalar.dma_start(win_sb[m][0:MT_ROWS[m], :],
                                    win_d[m * 128:m * 128 + MT_ROWS[m], :])

            qn = per.tile([128, S], F16, tag="qn")
            kfth = per.tile([128, S], F16, tag="kfth")
            kft2 = per.tile([128, S], F16, tag="kft2")
            qc = [per.tile([128, NCAP], F16, tag=f"qc{m}", name=f"qc{m}")
                  for m in range(2)]
            # per-head base-0 band copies (mixing lhsT base partitions
            # between back-to-back matmuls faults the PE)
            qb_t = [per.tile([32, S], F16, tag=f"qb{h}", name=f"qb{h}")
                    for h in range(1, 4)]
            kb_t = [per.tile([32, S], F16, tag=f"kb{h}", name=f"kb{h}")
                    for h in range(1, 4)]
            qcb_t = [per.tile([32, NCAP], F16, tag=f"qcb{h}",
                              name=f"qcb{h}") for h in range(1, 4)]
            # V tiles [j, (4 heads x 33)] (col 32 of each head block = 1):
            # aligned a0..a15 and shifted t1..t16 ([128i-64, 128i+64))
            va = [per.tile([128, HPC, 33], F16, tag=f"va{j}", name=f"va{j}")
                  for j in range(NT)]
            vs = [per.tile([128, HPC, 33], F16, tag=f"vs{j}", name=f"vs{j}")
                  for j in range(1, NT + 1)]
            sc_sb = [per.tile([128, S], F32, tag=f"sc{m}", name=f"sc{m}")
                     for m in range(NM)]
            al_sb = [per.tile([128, S], F16, tag=f"al{m}", name=f"al{m}")
                     for m in range(NM)]
            alT_sb = [per.tile([128, 1, NCAP], F16, tag=f"alT{j}",
                               name=f"alT{j}") for j in range(NT)]

            with (
                tc.tile_pool(name="pmm", bufs=4, space="PSUM") as pmm,
                tc.tile_pool(name="wrk", bufs=3) as wrk,
            ):
                # ---------- projections ----------
                for name, dst, wsel, lo in (
                    ("q", qn, wqc, 0), ("k", kfth, wk, 0),
                    ("k2", kft2, wk, 128),
                ):
                    for c in range(4):
                        sl = bass.ts(c, 512)
                        ps = pmm.tile([128, 512], F32, tag="mm")
                        for k in range(2):
                            nc.tensor.matmul(ps[:], wsel[k][:, lo:lo + 128],
                                             xh[k][:, sl],
                                             start=(k == 0), stop=(k == 1))
                        nc.vector.tensor_copy(dst[:, sl], ps[:])
                for m in range(2):
                    ps = pmm.tile([128, 512], F32, tag="mm")
                    for k in range(2):
                        nc.tensor.matmul(ps[:, 0:NCAP],
                                         wqc[k][:, m * 128:(m + 1) * 128],
                                         xch[k][:], start=(k == 0),
                                         stop=(k == 1))
                    nc.vector.tensor_copy(qc[m][:], ps[:, 0:NCAP])
                # V tiles; ones cols via Pool memset (SBUF)
                for idx, (vt, j0) in enumerate(
                        [(va[j], j * 128) for j in range(NT)] +
                        [(vs[j - 1], j * 128 - 64) for j in range(1, NT + 1)]):
                    lo = max(j0, 0)
                    hi = min(j0 + 128, S)
                    p0 = lo - j0
                    rows = hi - lo
                    ps = pmm.tile([128, 512], F32, tag="mm")
                    for k in range(2):
                        nc.tensor.matmul(ps[0:rows, 0:128], xh[k][:, lo:hi],
                                         wv[k][:], start=(k == 0),
                                         stop=(k == 1))
                    dstv = vt[p0:p0 + rows, :, 0:32]
                    srcv = ps[0:rows, 0:128].rearrange("p (h n) -> p h n", h=4)
                    if idx % 2 == 0:
                        nc.vector.tensor_copy(dstv, srcv)
                    else:
                        nc.scalar.activation(dstv, srcv, AF.Copy)
                    nc.gpsimd.memset(vt[:, :, 32:33], 1.0)

                for h in range(1, 4):
                    b0 = h * 32
                    nc.sync.dma_start(qb_t[h - 1][:], qn[b0:b0 + 32, :])
                    nc.sync.dma_start(kb_t[h - 1][:], kfth[b0:b0 + 32, :])
                    nc.sync.dma_start(qcb_t[h - 1][:],
                                      qc[0][b0:b0 + 32, :])

            if stage == 1:
                with tc.tile_pool(name="stub", bufs=1) as stub:
                    z = stub.tile([128, 2, S], F32, tag="z")
                    nc.vector.memset(z[:], 0.0)
                    nc.sync.dma_start(outT[:], z[:])

            # ---------- banded attention + interleaved routing ----------
            if stage >= 2:
             with (
                tc.tile_pool(name="pband", bufs=1, space="PSUM") as pb,
                tc.tile_pool(name="bwk", bufs=3) as bwk,
                tc.tile_pool(name="rwk", bufs=1) as rwk,
             ):
                def routing_scores(mt, c):
                    rows = MT_ROWS[mt]
                    msl = slice(mt * 128, mt * 128 + rows)
                    sl = bass.ts(c, 512)
                    ps = pb.tile([128, 512], F32, tag="rt", bufs=1)
                    nc.tensor.matmul(ps[0:rows, :], qc[0][:, msl],
                                     kfth[:, sl], start=True, stop=False)
                    nc.tensor.matmul(ps[0:rows, :], qc[1][:, msl],
                                     kft2[:, sl], start=False, stop=True)
                    nc.scalar.activation(sc_sb[mt][0:rows, sl],
                                         ps[0:rows, :], AF.Copy)

                def routing_thresh(mt):
                    rows = MT_ROWS[mt]
                    m8a = rwk.tile([128, 8], F32, tag="m8a")
                    m8b = rwk.tile([128, 8], F32, tag="m8b")
                    tmp = rwk.tile([128, S], F32, tag="mrtmp")
                    nc.vector.max(out=m8a[0:rows, :], in_=sc_sb[mt][0:rows, :])
                    nc.vector.match_replace(out=tmp[0:rows, :],
                                            in_to_replace=m8a[0:rows, :],
                                            in_values=sc_sb[mt][0:rows, :],
                                            imm_value=-1e30)
                    nc.vector.max(out=m8b[0:rows, :], in_=tmp[0:rows, :])
                    nc.vector.scalar_tensor_tensor(
                        out=al_sb[mt][0:rows, :], in0=sc_sb[mt][0:rows, :],
                        scalar=m8b[0:rows, 7:8], in1=win_sb[mt][0:rows, :],
                        op0=ALU.is_ge, op1=ALU.max)

                def routing_transpose(jt):
                    jsl = bass.ts(jt, 128)
                    ps = pb.tile([128, 1024], F16, tag="tr", bufs=1)
                    for mt in range(NM):
                        nc.tensor.transpose(ps[:, mt * 128:(mt + 1) * 128],
                                            al_sb[mt][:, jsl], ident[:])
                    if jt % 2 == 0:
                        nc.vector.tensor_copy(alT_sb[jt][:, 0, :],
                                              ps[:, 0:NCAP])
                    else:
                        nc.scalar.activation(alT_sb[jt][:, 0, :],
                                             ps[:, 0:NCAP], AF.Copy)

                routing_steps = (
                    [lambda m=m, c=c: routing_scores(m, c)
                     for m in range(NM) for c in range(4)] +
                    [lambda m=m: routing_thresh(m) for m in range(NM)] +
                    [lambda j=j: routing_transpose(j) for j in range(NT)]
                ) if stage >= 3 else []
                rstep = iter(routing_steps)

                bpart = int(os.environ.get("CGSA_BPART", "4"))
                for it in range(int(os.environ.get("CGSA_NT", NT))):
                    r0 = it * 128
                    isl = bass.ts(it, 128)
                    if it == 0:
                        subs = [(va[0], 0, 0, 128), (va[1], 128, 1, 128)]
                    elif it == NT - 1:
                        subs = [(vs[it - 1], r0 - 64, 2, 128),
                                (vs[it], r0 + 64, 3, 64)]
                    else:
                        subs = [(vs[it - 1], r0 - 64, 2, 128),
                                (vs[it], r0 + 64, 3, 128)]

                    av = pb.tile([128, 2, 256], F32, tag="av", bufs=2)
                    if True:
                        nc.tensor.matmul(
                            av[32:64, :, :].rearrange("p a b -> p (a b)"),
                            ones1[:, 0:32], ones1[:, 0:512],
                            start=True, stop=True, skip_group_check=True)
                    nsub = len(subs)
                    for si, (vt, j0, mc, jw) in enumerate(subs):
                        ps = pb.tile([128, HPC, 128], F32, tag="sc", bufs=2)
                        for h in range(HPC):
                            kt = kfth[0:32, :] if h == 0 else kb_t[h - 1][:]
                            qt = qn[0:32, :] if h == 0 else qb_t[h - 1][:]
                            nc.tensor.matmul(
                                ps[0:jw, h, :], kt[:, j0:j0 + jw],
                                qt[:, isl], start=(h == 0),
                                stop=False, skip_group_check=True)
                        nc.tensor.matmul(
                            ps[0:jw, :, :].rearrange("p a b -> p (a b)"),
                            ident[:, 0:jw],
                            mad[:, mc, :, :].rearrange("p a b -> p (a b)"),
                            start=False, stop=True, skip_group_check=True)
                        if bpart < 2:
                            continue
                        e = bwk.tile([128, HPC, 128], F16, tag="be")
                        nc.scalar.activation(e[0:jw, :, :], ps[0:jw, :, :],
                                             AF.Exp)
                        st = (si == 0)
                        sp = (si == nsub - 1)
                        for h in range(HPC):
                            p = h // 2
                            ob = (h % 2) * 64
                            nc.tensor.matmul(
                                av[ob:ob + 33, p, 0:128], vt[0:jw, h, :],
                                e[0:jw, h, :], start=(st and h < 2),
                                stop=(sp and h >= 2), skip_group_check=True)

                    if bpart < 3:
                        continue
                    r2i = bwk.tile([97, 2, 128], F16, tag="br2i")
                    with nc.allow_low_precision(reason="f16 recip bcast"):
                        nc.vector.reciprocal(r2i[:], av[0:97, :, 0:128])
                    r2w = bwk.tile([97, 2, 128], F16, tag="br2")
                    nc.vector.tensor_scalar(r2w[:], r2i[:], 60000.0, -60000.0,
                                            op0=ALU.min, op1=ALU.max)
                    rbp = pb.tile([97, 2, 256], F32, tag="rb", bufs=1)
                    for p in range(2):
                        nc.tensor.matmul(rbp[:, p, 0:128], selE[:],
                                         r2w[:, p, :], start=True, stop=True)
                    rbs = bwk.tile([97, 2, 128], F16, tag="brbs")
                    if it % 2 == 0:
                        nc.scalar.activation(rbs[:], rbp[:, :, 0:128],
                                             AF.Copy)
                    else:
                        nc.vector.tensor_copy(rbs[:], rbp[:, :, 0:128])
                    ctx = bwk.tile([97, 2, 128], F16, tag="bctx")
                    nc.vector.tensor_tensor(ctx[:], av[0:97, :, 0:128],
                                            rbs[:], op=ALU.mult)
                    pso = pb.tile([128, 2, 256], F32, tag="out", bufs=1)
                    for m in range(2):
                        msl = bass.ts(m, 128)
                        nc.tensor.matmul(pso[:, m, 0:128], woh[0][:, msl],
                                         ctx[:, 0, :], start=True, stop=False)
                        nc.tensor.matmul(pso[:, m, 0:128], woh[1][:, msl],
                                         ctx[:, 1, :], start=False, stop=True)
                    if bpart >= 4:
                        osb = bwk.tile([128, 2, 128], F32, tag="osb")
                        nc.scalar.activation(osb[:], pso[:, :, 0:128],
                                             AF.Copy)
                        nc.scalar.dma_start(outT[:, :, r0:r0 + 128], osb[:])

                    for _ in range(2):
                        step = next(rstep, None)
                        if step is not None:
                            step()
                for step in rstep:
                    step()

            # ---------- caller dense attention ----------
            if stage < 4:
                with tc.tile_pool(name="stub2", bufs=1) as stub2:
                    zc = stub2.tile([128, 2, NCAP], F32, tag="zc")
                    nc.vector.memset(zc[:], 0.0)
                    nc.sync.dma_start(outcT[:], zc[:])
            elif True:
             with tc.tile_pool(name="cacc", bufs=1, space="PSUM") as cacc:
                pCX = cacc.tile([128, 512], F32, tag="cavX")
                pCY = cacc.tile([128, 512], F32, tag="cavY")
                with (
                    tc.tile_pool(name="cps", bufs=2, space="PSUM") as cps,
                    tc.tile_pool(name="cwk", bufs=3) as cwk,
                ):
                    nc.tensor.matmul(pCX[32:64, :], ones1[:, 0:32],
                                     ones1[:, 0:512], start=True, stop=True,
                                     skip_group_check=True)
                    nc.tensor.matmul(pCY[32:64, :], ones1[:, 0:32],
                                     ones1[:, 0:512], start=True, stop=True,
                                     skip_group_check=True)
                    for jt in range(NT):
                        jsl = bass.ts(jt, 128)
                        psA = cps.tile([128, 2, 512], F32, tag="cscA",
                                       bufs=1)
                        psB = cps.tile([128, 2, 512], F32, tag="cscB",
                                       bufs=1)
                        for h in range(HPC):
                            kt = kfth[0:32, :] if h == 0 else kb_t[h - 1][:]
                            qt = (qc[0][0:32, :] if h == 0
                                  else qcb_t[h - 1][:])
                            psh = psA if h < 2 else psB
                            nc.tensor.matmul(psh[:, h % 2, 0:NCAP],
                                             kt[:, jsl], qt[:],
                                             start=True, stop=True)
                        eA = cwk.tile([128, 2, NCAP], F16, tag="ceA")
                        eB = cwk.tile([128, 2, NCAP], F16, tag="ceB")
                        nc.scalar.activation(eA[:], psA[:, :, 0:NCAP], AF.Exp)
                        nc.scalar.activation(eB[:], psB[:, :, 0:NCAP], AF.Exp)
                        em = cwk.tile([128, HPC, NCAP], F16, tag="cem")
                        bc2 = alT_sb[jt][:].to_broadcast((128, 2, NCAP))
                        nc.vector.tensor_tensor(em[:, 0:2, :], eA[:], bc2,
                                                op=ALU.mult)
                        nc.vector.tensor_tensor(em[:, 2:4, :], eB[:], bc2,
                                                op=ALU.mult)
                        st = (jt == 0)
                        sp = (jt == NT - 1)
                        for h in range(HPC):
                            pst = pCX if h < 2 else pCY
                            ob = (h % 2) * 64
                            nc.tensor.matmul(pst[ob:ob + 33, 0:NCAP],
                                             va[jt][:, h, :], em[:, h, :],
                                             start=st, stop=sp,
                                             skip_group_check=True)

                with (
                    tc.tile_pool(name="cep", bufs=1, space="PSUM") as cep,
                    tc.tile_pool(name="cwk2", bufs=1) as cwk2,
                ):
                    rcX = cwk2.tile([97, NCAP], F16, tag="crcX")
                    rcY = cwk2.tile([97, NCAP], F16, tag="crcY")
                    ri = cwk2.tile([97, 2, NCAP], F16, tag="cri")
                    with nc.allow_low_precision(reason="f16 recip bcast"):
                        nc.vector.reciprocal(ri[:, 0, :], pCX[0:97, 0:NCAP])
                        nc.vector.reciprocal(ri[:, 1, :], pCY[0:97, 0:NCAP])
                    nc.vector.tensor_scalar(rcX[:], ri[:, 0, :], 60000.0,
                                            -60000.0, op0=ALU.min,
                                            op1=ALU.max)
                    nc.vector.tensor_scalar(rcY[:], ri[:, 1, :], 60000.0,
                                            -60000.0, op0=ALU.min,
                                            op1=ALU.max)
                    rbc = cep.tile([97, 2, 512], F32, tag="crb")
                    nc.tensor.matmul(rbc[:, 0, 0:NCAP], selE[:], rcX[:],
                                     start=True, stop=True)
                    nc.tensor.matmul(rbc[:, 1, 0:NCAP], selE[:], rcY[:],
                                     start=True, stop=True)
                    rbcs = cwk2.tile([97, 2, NCAP], F16, tag="crbs")
                    nc.scalar.activation(rbcs[:], rbc[:, :, 0:NCAP], AF.Copy)
                    ctxX = cwk2.tile([97, NCAP], F16, tag="cctxX")
                    ctxY = cwk2.tile([97, NCAP], F16, tag="cctxY")
                    nc.vector.tensor_tensor(ctxX[:], pCX[0:97, 0:NCAP],
                                            rbcs[:, 0, :], op=ALU.mult)
                    nc.vector.tensor_tensor(ctxY[:], pCY[0:97, 0:NCAP],
                                            rbcs[:, 1, :], op=ALU.mult)
                    psoc = cep.tile([128, 2, 512], F32, tag="cout")
                    for m in range(2):
                        msl = bass.ts(m, 128)
                        nc.tensor.matmul(psoc[:, m, 0:NCAP], woh[0][:, msl],
                                         ctxX[:], start=True, stop=False)
                        nc.tensor.matmul(psoc[:, m, 0:NCAP], woh[1][:, msl],
                                         ctxY[:], start=False, stop=True)
                    osc = cwk2.tile([128, 2, NCAP], F32, tag="osc")
                    nc.vector.tensor_copy(osc[:], psoc[:, :, 0:NCAP])
                    nc.sync.dma_start(outcT[:], osc[:])

    nc.compile()
    nc.finalize()
    return nc


_NC_CACHE = None


def _get_program():
    global _NC_CACHE
    if _NC_CACHE is None:
        _NC_CACHE = _build_program(
            int(os.environ.get("CGSA_STAGE", "4")))
    return _NC_CACHE


def _np_fallback(x, Wq, bq, Wk, bk, Wv, bv, Wo, bo, opcode_types, pad_mask):
    """Exact numpy port of the reference (slow; only for inputs the fast
    path does not support: nonzero biases or non-trivial pad_mask)."""
    x = np.asarray(x, np.float32)
    b, s, d = x.shape
    scale = np.float32(SCALE)
    TOPK = 16

    def heads(t):
        return t.reshape(b, s, H, DK).transpose(0, 2, 1, 3)

    Q = heads((x @ Wq + bq).astype(np.float32))
    K = heads((x @ Wk + bk).astype(np.float32))
    V = heads((x @ Wv + bv).astype(np.float32))
    pos = np.arange(s)
    window = np.abs(pos[:, None] - pos[None, :]) <= WINDOW
    ms = np.einsum("bhid,bhjd->bij", Q, K).astype(np.float32) * (scale / H)
    topk_idx = np.argsort(-ms, axis=-1, kind="stable")[:, :, :TOPK]
    guided = np.zeros((b, s, s), bool)
    guided[np.arange(b)[:, None, None], np.arange(s)[None, :, None],
           topk_idx] = True
    attn = window[None] | (guided & (np.asarray(opcode_types) == 0)[:, :, None])
    out = np.empty((b, s, d), np.float32)
    for bb in range(b):
        ctx = np.empty((H, s, DK), np.float32)
        for h in range(H):
            sco = (Q[bb, h] @ K[bb, h].T) * scale
            sco = np.where(attn[bb], sco, -1e9)
            sco = np.where(np.asarray(pad_mask)[bb][None, :] != 0, sco, -1e9)
            e = np.exp(sco - sco.max(axis=-1, keepdims=True))
            w = np.nan_to_num(e / e.sum(axis=-1, keepdims=True))
            ctx[h] = w @ V[bb, h]
        out[bb] = ctx.transpose(1, 0, 2).reshape(s, d)
    return out @ Wo + bo


def _host_prepare(x, Wq, Wk, Wv, Wo, opcode_types):
    x = np.ascontiguousarray(np.asarray(x, np.float32))
    Wq = np.asarray(Wq, np.float32) * SCALE
    Wk = np.asarray(Wk, np.float32)
    Wv = np.asarray(Wv, np.float32)
    Wo = np.asarray(Wo, np.float32)
    opcode = np.asarray(opcode_types)

    # additive banded masks [p_j, class, q_i]; class offsets j0-r0:
    # 0 -> 0, 1 -> +128, 2 -> -64, 3 -> +64
    mad = np.full((128, 4, HPC, 128), NEGM, np.float16)
    pj = np.arange(128)[:, None]
    qi = np.arange(128)[None, :]
    for c, off in enumerate((0, 128, -64, 64)):
        m = np.where(np.abs(off + pj - qi) <= WINDOW, np.float16(0.0),
                     np.float16(NEGM))
        mad[:, c, :, :] = m[:, None, :]

    selE = np.zeros((97, 97), np.float16)
    selE[32, 0:32] = 1.0
    selE[96, 64:96] = 1.0
    ident = np.eye(128, dtype=np.float16)

    in_maps = []
    meta = []
    for b in range(B):
        cidx = np.where(opcode[b] == 0)[0]
        nrows = len(cidx)
        if nrows > NCAP:
            raise RuntimeError(f"caller rows {nrows} exceed capacity {NCAP}")
        xc = np.zeros((NCAP, D), np.float32)
        xc[:nrows] = x[b, cidx]
        ci = np.full((NCAP,), -1e6, np.float64)
        ci[:nrows] = cidx
        win = (np.abs(ci[:, None] - np.arange(S)[None, :]) <= WINDOW)
        win = win.astype(np.float16)
        meta.append((cidx, nrows))
        for hg in range(2):
            own = np.arange(hg * DH, (hg + 1) * DH)
            rest = np.setdiff1d(np.arange(D), own)
            perm = np.concatenate([own, rest])
            woh_arr = np.zeros((2, 97, D), np.float32)
            for p in range(2):
                woh_arr[p, 0:32] = Wo[own[p * 64:p * 64 + 32]]
                woh_arr[p, 64:96] = Wo[own[p * 64 + 32:p * 64 + 64]]
            in_maps.append({
                "xTh": np.ascontiguousarray(
                    x[b].T.reshape(2, 128, S).astype(np.float16)),
                "xcTh": np.ascontiguousarray(
                    xc.T.reshape(2, 128, NCAP).astype(np.float16)),
                "wqc": np.ascontiguousarray(
                    Wq[:, perm].reshape(2, 128, D).astype(np.float16)),
                "wk": np.ascontiguousarray(
                    Wk[:, perm].reshape(2, 128, D).astype(np.float16)),
                "wv": np.ascontiguousarray(
                    Wv[:, own].reshape(2, 128, HPC * 32).astype(np.float16)),
                "woh": woh_arr.astype(np.float16),
                "selE": selE,
                "ident": ident,
                "mad": mad,
                "win": win,
            })
    return in_maps, meta


def _assemble(results, meta, bo):
    bo = np.asarray(bo, np.float32)
    out = np.empty((B, S, D), np.float32)
    for b in range(B):
        cidx, nrows = meta[b]
        full = (results[2 * b]["outT"] + results[2 * b + 1]["outT"])
        full = full.transpose(1, 0, 2).reshape(D, S).T
        if nrows > 0:
            oc = (results[2 * b]["outcT"] + results[2 * b + 1]["outcT"])
            oc = oc.transpose(1, 0, 2).reshape(D, NCAP).T[:nrows]
            full = full.copy()
            full[cidx] = oc
        out[b] = full + bo[None, :]
    return out


def kernel(x, Wq, bq, Wk, bk, Wv, bv, Wo, bo, opcode_types, pad_mask,
           _trace=False):
    if (np.any(np.asarray(bq)) or np.any(np.asarray(bk)) or
            np.any(np.asarray(bv)) or np.any(np.asarray(pad_mask) == 0)):
        return _np_fallback(x, Wq, bq, Wk, bk, Wv, bv, Wo, bo,
                            opcode_types, pad_mask)
    nc = _get_program()
    in_maps, meta = _host_prepare(x, Wq, Wk, Wv, Wo, opcode_types)
    res = run_bass_kernel_spmd(nc, in_maps, core_ids=list(range(8)),
                               trace=_trace)
    out = _assemble(res.results, meta, bo)
    if _trace:
        kernel.last_exec_time_ns = res.exec_time_ns
        kernel.last_results = res
    return out
